# revision 4
# baseline (speedup 1.0000x reference)
"""Trainium2 Bass kernel for nn_Attention_4398046511861.

Bahdanau-style attention:
    proj_e = einsum('sbe,ae->sba', enc, w_ae) + b_ae
    proj_d = einsum('bd,ad->ba', dec, w_ad) + b_ad
    scores = einsum('sba,ba->sb', proj_e, proj_d)
    alphas = softmax(scores, axis=0)          # over sequence
    out    = einsum('sb,sbe->be', alphas, enc)

Key algebraic rewrite: scores[s,b] = enc[s,b,:] @ v_b + const_b where
v_b = w_ae^T @ proj_d[b] and const_b = b_ae . proj_d[b].  const_b is
uniform over s, so it cancels in the softmax and is dropped.  This
turns the dominant [S,B,E]x[A,E] projection into a per-batch matvec and
makes the kernel purely memory bound (one streaming read of enc).

Sharding: data-parallel over batch, B=32 -> 4 batches per core x 8 cores.
enc is shipped as fp16 (randn data, no range issues; 11-bit mantissa).

Per-core device program (natural layout [s_partition, e_free], all of
the core's enc slice stays SBUF-resident):
  - prologue: proj_d via PE (contraction over D on partitions),
    v rows via PE, broadcast to all partitions via GPSIMD.
  - scores: DVE tensor_tensor_reduce (fused multiply + free-dim reduce)
    against the broadcast v -> scores land [s_part, s_chunk].
  - softmax: DVE reduce_max + GPSIMD partition_all_reduce(max),
    ACT Exp(bias=-max) with fused accum_out row-sum,
    GPSIMD partition_all_reduce(add) -> L.
  - context: PE matmuls, alpha column as stationary [128,1], enc tile
    moving, accumulated over the 16 s-chunks in PSUM; ACT scales by 1/L.
"""

import os
import numpy as np

import concourse.bass as bass
import concourse.tile as tile
from concourse import bacc, mybir
from concourse import bass_isa
from concourse.bass_utils import run_bass_kernel_spmd

F32 = mybir.dt.float32

S, B, E, A, D = 2048, 32, 1024, 128, 1024
NCORES = 8
BLOC = B // NCORES          # 4 batches per core
SCH = 128                   # sequence positions per chunk (partition dim)
NSCH = S // SCH             # 16 s-chunks
QCH = 4                     # s-chunks per DMA supertile
NQ = NSCH // QCH            # 4 supertiles per batch

ENC_DT = mybir.dt.float16
ENC_NP = np.float16


def build_kernel(enc_dt=ENC_DT):
    nc = bacc.Bacc("TRN2", debug=False)

    enc = nc.dram_tensor("enc", [S, BLOC, E], enc_dt, kind="ExternalInput").ap()
    dec_t = nc.dram_tensor("dec_t", [D, BLOC], F32, kind="ExternalInput").ap()
    w_ad_t = nc.dram_tensor("w_ad_t", [D, A], F32, kind="ExternalInput").ap()
    w_ae_in = nc.dram_tensor("w_ae", [A, E], F32, kind="ExternalInput").ap()
    b_ad_in = nc.dram_tensor("b_ad", [A, 1], F32, kind="ExternalInput").ap()
    out = nc.dram_tensor("out", [BLOC, E], F32, kind="ExternalOutput").ap()

    from contextlib import ExitStack

    with tile.TileContext(nc) as tc:
        with ExitStack() as ctx:
            singles = ctx.enter_context(tc.tile_pool(name="singles", bufs=1))
            encp = ctx.enter_context(tc.tile_pool(name="encp", bufs=BLOC * NQ))
            scr = ctx.enter_context(tc.tile_pool(name="scr", bufs=3))
            ttrout = ctx.enter_context(tc.tile_pool(name="ttrout", bufs=2))
            pps = ctx.enter_context(tc.tile_pool(name="pps", bufs=1, space="PSUM"))
            pctx = ctx.enter_context(tc.tile_pool(name="pctx", bufs=2, space="PSUM"))

            # ---- weight / decoder loads -------------------------------------
            w_ae_sb = singles.tile([A, E], F32)
            nc.sync.dma_start(out=w_ae_sb, in_=w_ae_in)
            w_ad_sb = singles.tile([128, D // 128, A], F32)
            nc.sync.dma_start(out=w_ad_sb, in_=w_ad_t.rearrange("(c p) a -> p c a", p=128))
            dec_sb = singles.tile([128, D // 128, BLOC], F32)
            nc.sync.dma_start(out=dec_sb, in_=dec_t.rearrange("(c p) b -> p c b", p=128))
            b_ad_sb = singles.tile([A, 1], F32)
            nc.sync.dma_start(out=b_ad_sb, in_=b_ad_in)

            # ---- proj_d [A, BLOC] = w_ad @ dec^T + b_ad ---------------------
            projd_ps = pps.tile([A, BLOC], F32, tag="projd")
            nd = D // 128
            for c in range(nd):
                nc.tensor.matmul(
                    projd_ps,
                    w_ad_sb[:, c, :],
                    dec_sb[:, c, :],
                    start=(c == 0),
                    stop=(c == nd - 1),
                )
            projd_sb = singles.tile([A, BLOC], F32)
            nc.vector.tensor_scalar_add(projd_sb, projd_ps, b_ad_sb)

            # ---- v_b rows and their partition-broadcast ---------------------
            v_rep = []
            for b in range(BLOC):
                vps = pps.tile([1, E], F32, tag="vps")
                for h in range(2):
                    nc.tensor.matmul(
                        vps[:, h * 512 : (h + 1) * 512],
                        projd_sb[:, b : b + 1],
                        w_ae_sb[:, h * 512 : (h + 1) * 512],
                        start=True,
                        stop=True,
                    )
                vrow = singles.tile([1, E], enc_dt, tag=f"vrow{b}")
                nc.scalar.copy(out=vrow, in_=vps)
                vr = singles.tile([128, E], enc_dt, tag=f"vrep{b}")
                nc.gpsimd.partition_broadcast(vr, vrow, channels=128)
                v_rep.append(vr)

            # ---- main per-batch pipeline ------------------------------------
            for b in range(BLOC):
                etiles = []
                sc = scr.tile([128, NSCH], F32, tag="scores")
                for q in range(NQ):
                    et = encp.tile([128, QCH, E], enc_dt, tag="enc")
                    nc.sync.dma_start(
                        out=et,
                        in_=enc[q * 512 : (q + 1) * 512, b : b + 1, :].rearrange(
                            "(c p) o e -> p c (o e)", p=128
                        ),
                    )
                    etiles.append(et)
                    for c in range(QCH):
                        j = q * QCH + c
                        tout = ttrout.tile([128, E], enc_dt, tag="ttr")
                        nc.vector.affine_mul_reduce(
                            tout,
                            sc[:, j : j + 1],
                            et[:, c, :],
                            v_rep[b],
                            scale=1.0,
                            bias=0.0,
                        )

                # softmax over all 2048 scores of this batch
                rmax = scr.tile([128, 1], F32, tag="rmax")
                nc.vector.reduce_max(out=rmax, in_=sc, axis=mybir.AxisListType.X)
                gmax = scr.tile([128, 1], F32, tag="gmax")
                nc.gpsimd.partition_all_reduce(gmax, rmax, 128, bass_isa.ReduceOp.max)
                negmax = scr.tile([128, 1], F32, tag="negmax")
                nc.vector.tensor_scalar_mul(negmax, gmax, -1.0)
                al = scr.tile([128, NSCH], enc_dt, tag="alpha")
                rowsum = scr.tile([128, 1], F32, tag="rowsum")
                nc.scalar.activation(
                    out=al,
                    in_=sc,
                    func=mybir.ActivationFunctionType.Exp,
                    bias=negmax,
                    scale=1.0,
                    accum_out=rowsum,
                )
                lsum = scr.tile([128, 1], F32, tag="lsum")
                nc.gpsimd.partition_all_reduce(lsum, rowsum, 128, bass_isa.ReduceOp.add)
                linv = scr.tile([128, 1], F32, tag="linv")
                nc.vector.reciprocal(linv, lsum)

                # context[e] = sum_s alpha[s] * enc[s, e], accumulated in PSUM
                cps = [
                    pctx.tile([1, 512], F32, tag=f"cps{h}", name=f"cps{h}")
                    for h in range(2)
                ]
                for q in range(NQ):
                    for c in range(QCH):
                        j = q * QCH + c
                        for h in range(2):
                            nc.tensor.matmul(
                                cps[h],
                                al[:, j : j + 1],
                                etiles[q][:, c, h * 512 : (h + 1) * 512],
                                start=(j == 0),
                                stop=(j == NSCH - 1),
                            )

                ob = scr.tile([1, E], F32, tag="outrow")
                for h in range(2):
                    nc.scalar.activation(
                        out=ob[:, h * 512 : (h + 1) * 512],
                        in_=cps[h],
                        func=mybir.ActivationFunctionType.Copy,
                        bias=0.0,
                        scale=linv[0:1, :],
                    )
                nc.scalar.dma_start(out=out[b : b + 1, :], in_=ob)

    nc.compile()
    return nc


_NC_CACHE = {}


def _get_nc():
    if "nc" not in _NC_CACHE:
        _NC_CACHE["nc"] = build_kernel()
    return _NC_CACHE["nc"]


def make_in_maps(enc_outputs, dec_output, w_ae, w_ad, b_ad):
    enc16 = np.asarray(enc_outputs, dtype=np.float32).astype(ENC_NP)
    dec = np.asarray(dec_output, dtype=np.float32)
    w_ad_t = np.ascontiguousarray(np.asarray(w_ad, dtype=np.float32).T)
    w_ae_c = np.ascontiguousarray(np.asarray(w_ae, dtype=np.float32))
    b_ad_c = np.asarray(b_ad, dtype=np.float32).reshape(A, 1)
    in_maps = []
    for core in range(NCORES):
        b0 = core * BLOC
        in_maps.append(
            {
                "enc": np.ascontiguousarray(enc16[:, b0 : b0 + BLOC, :]),
                "dec_t": np.ascontiguousarray(dec[b0 : b0 + BLOC, :].T),
                "w_ad_t": w_ad_t,
                "w_ae": w_ae_c,
                "b_ad": b_ad_c,
            }
        )
    return in_maps


def kernel(enc_outputs, dec_output, w_ae, b_ae, w_ad, b_ad, _trace=False):
    """Full-input / full-output entry point.  b_ae is algebraically inert
    (uniform shift over the softmax axis) and is ignored."""
    nc = _get_nc()
    in_maps = make_in_maps(enc_outputs, dec_output, w_ae, w_ad, b_ad)
    res = run_bass_kernel_spmd(nc, in_maps, core_ids=list(range(NCORES)), trace=_trace)
    out = np.concatenate([r["out"] for r in res.results], axis=0)
    if _trace:
        return out, res
    return out
